# revision 2
# baseline (speedup 1.0000x reference)
"""Trainium2 Bass kernel v2 for nn_Critic (8-agent attention critic).

Data-parallel over batch across 8 cores; BN folded into first-layer weights
on host (batch stats depend only on inputs). Changes vs v1:
  - Lrelu (alpha=0.01) + bias fused into the PSUM-evacuation activations
    (kills all DVE lrelu traffic).
  - q,k evacuated in one [E,2CH] activation.
  - delta-stage: ONE matmul per 2-j quad (shared delta lhsT), computing all
    8 j incl. the j==i garbage block (f1 skips it) -> plain adjacent APs.
  - m-stage routes per quad: DVE-direct from PSUM (1x), or ACT evac + DVE
    2x, or ACT evac + Pool TT (3rd engine lane).
  - msk = f2_psum * onehot directly on DVE; bf2[action] added on host.
  - All host tensors repacked so each chunk needs 3 input DMAs and the
    weights load in ~9 DMAs (HWDGE descriptor-gen is serialized at
    ~625ns/instruction).
"""
import sys

sys.path.insert(0, "/opt/trn_rl_repo")

import numpy as np
import ml_dtypes

import concourse.bass as bass
import concourse.mybir as mybir
import concourse.tile as tile
from concourse import bacc
from concourse.alu_op_type import AluOpType
from concourse.bass_utils import run_bass_kernel_spmd

BF16 = mybir.dt.bfloat16
F32 = mybir.dt.float32
FP8 = mybir.dt.float8e4
AF = mybir.ActivationFunctionType

A, B, OBS, ACT, E, H = 8, 32768, 128, 32, 128, 4
FP8_SCALE = 16.0
D = E // H
NCORES = 8
EPS = 1e-5
SLOPE = 0.01

DEF_CFG = {
    "bufs": {"pp": 3, "mm": 3, "xo": 2, "gsb": 4, "est": 4, "h1": 3,
             "f2": 3, "psA": 3, "psB": 4},
    # per-j route cycle: "dve" = TT direct from PSUM (1x); "act_dve" = ACT
    # evac + DVE 2x; "act_pool" = ACT evac + Pool TT.
    "m_sched": ("dve", "act_pool", "dve", "dve", "act_pool", "dve",
                "dve", "act_pool"),
    "mi_sched": ("dve",),
    "v_path": "act",      # "act" (Lrelu act) | "dve" (scalar_tensor_tensor)
    "per_j": True,        # per-j 1-bank pG tiles (deeper psB ring)
    "msk_path": "dve",    # "dve" only for now
    "prod_path": "dve",   # h1*w2oh product: "pool" | "dve"
    "p_gran": 4,          # j-blocks per P-stage DVE op
    "qk_path": "act",     # q/k evacuation: "act" | "dve"
    "f1_fp8": True,       # m_all in fp8e4 + DoubleRow f1 matmuls
}


def build_nc(Bs, CH, cfg=None):
    cfg = dict(DEF_CFG, **(cfg or {}))
    bf = cfg["bufs"]
    NCH = Bs // CH
    nc = bacc.Bacc(None, target_bir_lowering=False, debug=False)

    obs_e = nc.declare_dram_parameter("obs_T", [OBS, A, Bs], BF16, isOutput=False)
    act_e = nc.declare_dram_parameter("act_T", [2 * ACT, 4, Bs], BF16, isOutput=False)
    w2_e = nc.declare_dram_parameter("w2oh_T", [E, A, Bs], BF16, isOutput=False)
    wgo_e = nc.declare_dram_parameter("wgo", [OBS, A * E], BF16, isOutput=False)
    wga_e = nc.declare_dram_parameter("wga", [2 * ACT, A * E], BF16, isOutput=False)
    ws_e = nc.declare_dram_parameter("ws", [OBS, A * E], BF16, isOutput=False)
    wqkv_e = nc.declare_dram_parameter("wqkv", [E, 3 * E], BF16, isOutput=False)
    wf1_e = nc.declare_dram_parameter("wf1", [E, 2 * A * E], BF16, isOutput=False)
    wf1dr_e = nc.declare_dram_parameter("wf1dr", [E, 2 * A * E], mybir.dt.float8e4, isOutput=False)
    delta_e = nc.declare_dram_parameter("delta", [E, E], BF16, isOutput=False)
    osel_e = nc.declare_dram_parameter("onescol", [E, A * A], BF16, isOutput=False)
    bias_e = nc.declare_dram_parameter("bias_all", [E, 3 * A], F32, isOutput=False)
    out_e = nc.declare_dram_parameter("out", [A, Bs], F32, isOutput=True)

    with tile.TileContext(nc) as tc:
        with (
            tc.tile_pool(name="wpool", bufs=1) as wp,
            tc.tile_pool(name="xin", bufs=bf.get("xin", 3)) as xin_p,
            tc.tile_pool(name="est", bufs=bf.get("est", 3)) as e_p,
            tc.tile_pool(name="st", bufs=bf.get("st", 2)) as st_p,
            tc.tile_pool(name="pp", bufs=bf.get("pp", 2)) as pp_p,
            tc.tile_pool(name="mm", bufs=bf.get("mm", 2)) as mm_p,
            tc.tile_pool(name="gsb", bufs=bf.get("gsb", 3)) as g_p,
            tc.tile_pool(name="h1", bufs=bf.get("h1", 2)) as h1_p,
            tc.tile_pool(name="f2", bufs=bf.get("f2", 2)) as f2_p,
            tc.tile_pool(name="orow", bufs=bf.get("orow", 2)) as orow_p,
            tc.tile_pool(name="psA", bufs=bf.get("psA", 3), space="PSUM") as psA,
            tc.tile_pool(name="psB", bufs=bf.get("psB", 2), space="PSUM") as psB,
            tc.tile_pool(name="psR", bufs=1, space="PSUM") as psR,
        ):
            # ---- load weights once (merged DMAs) ----
            wgo_t = wp.tile([OBS, A * E], BF16)
            wga_t = wp.tile([2 * ACT, A * E], BF16)
            ws_t = wp.tile([OBS, A * E], BF16)
            wqkv_t = wp.tile([E, 3 * E], BF16)
            wf1_t = wp.tile([E, 2 * A * E], BF16)
            wf1dr_t = wp.tile([E, 2 * A * E], FP8)
            delta_t = wp.tile([E, E], BF16)
            osel_t = wp.tile([E, A * A], BF16)
            bias_t = wp.tile([E, 3 * A], F32)
            ones_t = wp.tile([E, 1], BF16)
            nc.sync.dma_start(wgo_t[:], wgo_e[:])
            nc.sync.dma_start(wga_t[:], wga_e[:])
            nc.sync.dma_start(ws_t[:], ws_e[:])
            nc.sync.dma_start(wqkv_t[:], wqkv_e[:])
            nc.sync.dma_start(wf1_t[:], wf1_e[:])
            if cfg["f1_fp8"]:
                nc.sync.dma_start(wf1dr_t[:], wf1dr_e[:])
            nc.sync.dma_start(delta_t[:], delta_e[:])
            nc.sync.dma_start(osel_t[:], osel_e[:])
            nc.sync.dma_start(bias_t[:], bias_e[:])
            nc.vector.memset(ones_t[:], 1.0)

            def wq_l():
                return wqkv_t[:, 0:E]

            def wk_l():
                return wqkv_t[:, E:2 * E]

            def wv_l():
                return wqkv_t[:, 2 * E:3 * E]

            def wf1x(a):
                return wf1_t[:, a * E:(a + 1) * E]

            def wf1s(a):
                return wf1_t[:, (A + a) * E:(A + a + 1) * E]

            def bg(a):
                return bias_t[:, a:a + 1]

            def bs(a):
                return bias_t[:, A + a:A + a + 1]

            def bh1(a):
                return bias_t[:, 2 * A + a:2 * A + a + 1]

            mcnt = [0]
            micnt = [0]

            def emit_dma(ch):
                c0 = ch * CH
                xo_all = xin_p.tile([OBS, A * CH], BF16, tag="xo", bufs=bf.get("xo"))
                xa_all = xin_p.tile([2 * ACT, 4 * CH], BF16, tag="xa")
                w2_all = xin_p.tile([E, A * CH], BF16, tag="w2")
                nc.sync.dma_start(
                    xo_all[:].rearrange("p (a b) -> p a b", a=A),
                    obs_e[:, :, c0:c0 + CH])
                nc.sync.dma_start(
                    xa_all[:].rearrange("p (g b) -> p g b", g=4),
                    act_e[:, :, c0:c0 + CH])
                nc.sync.dma_start(
                    w2_all[:].rearrange("p (a b) -> p a b", a=A),
                    w2_e[:, :, c0:c0 + CH])
                return {
                    "xo": xo_all, "xa": xa_all, "w2": w2_all,
                    "s": st_p.tile([E, A * CH], BF16, tag="s_st", name="s_st"),
                    "qk": st_p.tile([E, 2 * A * CH], BF16, tag="qk_st",
                                    name="qk_st"),
                    "v": st_p.tile([E, A * CH], BF16, tag="v_st", name="v_st"),
                }

            def stage_A(S, a, warm=False):
                asl = slice(a * CH, (a + 1) * CH)
                x_o = S["xo"][:, asl]
                x_a = S["xa"][32 * (a % 2):32 * (a % 2) + 32,
                              (a // 2) * CH:(a // 2 + 1) * CH]
                ps_e = psA.tile([E, CH], F32, tag="ps")
                nc.tensor.matmul(ps_e[:], wgo_t[:, a * E:(a + 1) * E], x_o,
                                 start=True, stop=False)
                nc.tensor.matmul(
                    ps_e[:],
                    wga_t[32 * (a % 2):32 * (a % 2) + 32, a * E:(a + 1) * E],
                    x_a, start=False, stop=True)
                e_t = e_p.tile([E, CH], BF16, tag="e_t")
                nc.scalar.activation(e_t[:], ps_e[:], AF.Lrelu,
                                     bias=bg(a), alpha=SLOPE)
                ps_s = psA.tile([E, CH], F32, tag="ps")
                nc.tensor.matmul(ps_s[:], ws_t[:, a * E:(a + 1) * E], x_o,
                                 start=True, stop=True)
                nc.scalar.activation(S["s"][:, asl], ps_s[:], AF.Lrelu,
                                     bias=bs(a), alpha=SLOPE)
                if cfg["per_j"]:
                    ps_q = psA.tile([E, CH], F32, tag="ps")
                    nc.tensor.matmul(ps_q[:], wq_l(), e_t[:],
                                     start=True, stop=True)
                    qdst = S["qk"][:, asl]
                    if warm or cfg["qk_path"] == "dve":
                        nc.vector.tensor_tensor(
                            qdst, ps_q[:],
                            ones_t[:, 0:1].broadcast_to([E, CH]),
                            AluOpType.mult)
                    else:
                        nc.scalar.activation(qdst, ps_q[:], AF.Identity)
                    ps_k = psA.tile([E, CH], F32, tag="ps")
                    nc.tensor.matmul(ps_k[:], wk_l(), e_t[:],
                                     start=True, stop=True)
                    kdst = S["qk"][:, A * CH + a * CH:A * CH + (a + 1) * CH]
                    if warm or cfg["qk_path"] == "dve":
                        nc.vector.tensor_tensor(
                            kdst, ps_k[:],
                            ones_t[:, 0:1].broadcast_to([E, CH]),
                            AluOpType.mult)
                    else:
                        nc.scalar.activation(kdst, ps_k[:], AF.Identity)
                else:
                    ps_qk = psB.tile([E, 2 * CH], F32, tag="ps2")
                    nc.tensor.matmul(ps_qk[:, 0:CH], wq_l(), e_t[:],
                                     start=True, stop=True)
                    nc.tensor.matmul(ps_qk[:, CH:2 * CH], wk_l(), e_t[:],
                                     start=True, stop=True)
                    qk_dst = S["qk"][:].rearrange(
                        "p (t a b) -> p t a b", t=2, a=A)[:, :, a, :]
                    nc.scalar.activation(qk_dst, ps_qk[:].rearrange(
                        "p (t b) -> p t b", t=2), AF.Identity)
                ps_v = psA.tile([E, CH], F32, tag="ps")
                nc.tensor.matmul(ps_v[:], wv_l(), e_t[:],
                                 start=True, stop=True)
                if cfg["v_path"] == "act":
                    nc.scalar.activation(S["v"][:, asl], ps_v[:], AF.Lrelu,
                                         alpha=SLOPE)
                else:
                    nc.vector.scalar_tensor_tensor(
                        S["v"][:, asl], ps_v[:], SLOPE, ps_v[:],
                        AluOpType.mult, AluOpType.max)

            def stage_P(S, i):
                isl = slice(i * CH, (i + 1) * CH)
                koff = A * CH
                gp = cfg["p_gran"]          # j-blocks per DVE op (8/4/2)
                P_all = pp_p.tile([E, A * CH], BF16)
                for h0 in range(0, A, gp):
                    hsl = slice(h0 * CH, (h0 + gp) * CH)
                    q_rep = S["qk"][:, None, isl].broadcast_to([E, gp, CH])
                    ksl = slice(koff + h0 * CH, koff + (h0 + gp) * CH)
                    nc.vector.tensor_tensor(
                        P_all[:, hsl].rearrange("p (j b) -> p j b", j=gp),
                        q_rep,
                        S["qk"][:, ksl].rearrange("p (j b) -> p j b", j=gp),
                        AluOpType.mult)
                return P_all

            def consume_quad(S, route, m_all, pG, dst_sl, v_sl, width):
                gsl = slice(0, width * CH)
                if route == "dve":
                    nc.vector.tensor_tensor(m_all[:, dst_sl], pG[:, gsl],
                                            S["v"][:, v_sl], AluOpType.mult)
                else:
                    g_sb = g_p.tile([E, 2 * CH], BF16, tag="g_sb")
                    nc.scalar.activation(g_sb[:, gsl], pG[:, gsl],
                                         AF.Identity)
                    eng = nc.gpsimd if route == "act_pool" else nc.vector
                    eng.tensor_tensor(m_all[:, dst_sl], g_sb[:, gsl],
                                      S["v"][:, v_sl], AluOpType.mult)

            def consume_j(S, route, m_all, pG, jsl):
                if route == "dve":
                    nc.vector.tensor_tensor(m_all[:, jsl], pG[:],
                                            S["v"][:, jsl], AluOpType.mult)
                else:
                    g_sb = g_p.tile([E, CH], BF16, tag="g_sb1")
                    nc.scalar.activation(g_sb[:], pG[:], AF.Identity)
                    eng = nc.gpsimd if route == "act_pool" else nc.vector
                    eng.tensor_tensor(m_all[:, jsl], g_sb[:],
                                      S["v"][:, jsl], AluOpType.mult)

            def stage_m_perj(S, i, P_all, drain=False):
                m_dt = FP8 if cfg["f1_fp8"] else BF16
                m_all = mm_p.tile([E, A * CH], m_dt)
                sched = (("dve", "act_pool") if drain else cfg["m_sched"])
                for j in range(A):
                    if j == i:
                        continue
                    jsl = slice(j * CH, (j + 1) * CH)
                    pG = psB.tile([E, CH], F32, tag="ps2")
                    nc.tensor.matmul(pG[:], delta_t[:], P_all[:, jsl],
                                     start=True, stop=True)
                    route = sched[mcnt[0] % len(sched)]
                    mcnt[0] += 1
                    consume_j(S, route, m_all, pG, jsl)
                return m_all

            def stage_m(S, i, P_all, drain=False):
                if cfg["per_j"]:
                    return stage_m_perj(S, i, P_all, drain)
                m_all = mm_p.tile([E, A * CH], BF16)
                for qd in range(4):
                    j0 = 2 * qd
                    qsl = slice(j0 * CH, (j0 + 2) * CH)
                    if i == j0 or i == j0 + 1:
                        j = j0 + 1 if i == j0 else j0
                        jsl = slice(j * CH, (j + 1) * CH)
                        pG = psB.tile([E, 2 * CH], F32, tag="ps2")
                        nc.tensor.matmul(pG[:, 0:CH], delta_t[:],
                                         P_all[:, jsl],
                                         start=True, stop=True)
                        route = cfg["mi_sched"][micnt[0] % len(cfg["mi_sched"])]
                        micnt[0] += 1
                        consume_quad(S, route, m_all, pG, jsl, jsl, 1)
                    else:
                        pG = psB.tile([E, 2 * CH], F32, tag="ps2")
                        for jj in range(2):
                            nc.tensor.matmul(
                                pG[:, jj * CH:(jj + 1) * CH], delta_t[:],
                                P_all[:, (j0 + jj) * CH:(j0 + jj + 1) * CH],
                                start=True, stop=True)
                        route = cfg["m_sched"][mcnt[0] % len(cfg["m_sched"])]
                        mcnt[0] += 1
                        consume_quad(S, route, m_all, pG, qsl, qsl, 2)
                return m_all

            def stage_f(S, i, m_all, prow):
                isl = slice(i * CH, (i + 1) * CH)
                ph = psA.tile([E, CH], F32, tag="ps")
                if cfg["f1_fp8"]:
                    runs = [r for r in ([list(range(0, i)),
                                         list(range(i + 1, A))]) if r]
                    first = True
                    singles = []
                    lhs_dr = wf1dr_t[:, 2 * i * E:(2 * i + 2) * E].rearrange(
                        "p (t m) -> p t m", t=2)
                    lhs_1 = wf1dr_t[:, 2 * i * E:(2 * i + 1) * E]
                    for run in runs:
                        while len(run) >= 2:
                            j = run.pop(0)
                            run.pop(0)
                            rhs = m_all[:, j * CH:(j + 2) * CH].rearrange(
                                "p (t b) -> p t b", t=2)
                            nc.tensor.matmul(
                                ph[:], lhs_dr, rhs, start=first, stop=False,
                                perf_mode=mybir.MatmulPerfMode.DoubleRow)
                            first = False
                        singles.extend(run)
                    for j in singles:
                        nc.tensor.matmul(ph[:], lhs_1,
                                         m_all[:, j * CH:(j + 1) * CH],
                                         start=first, stop=False)
                        first = False
                else:
                    others = [j for j in range(A) if j != i]
                    for nj, j in enumerate(others):
                        nc.tensor.matmul(ph[:], wf1x(i),
                                         m_all[:, j * CH:(j + 1) * CH],
                                         start=(nj == 0), stop=False)
                nc.tensor.matmul(ph[:], wf1s(i), S["s"][:, isl],
                                 start=False, stop=True)
                h1_t = h1_p.tile([E, CH], BF16, tag="h1")
                nc.scalar.activation(h1_t[:], ph[:], AF.Lrelu,
                                     bias=bh1(i), alpha=SLOPE)
                prod = f2_p.tile([E, CH], BF16, tag="prod")
                peng = nc.gpsimd if cfg["prod_path"] == "pool" else nc.vector
                peng.tensor_tensor(prod[:], h1_t[:], S["w2"][:, isl],
                                   AluOpType.mult)
                nc.tensor.matmul(prow[:], osel_t[:, i * A:(i + 1) * A],
                                 prod[:], start=(i == 0), stop=(i == A - 1))

            # ---- software-pipelined superloop: A(ch) overlaps I(ch-1) ----
            cur = prev = None
            prow = None
            pend_P = {}
            pend_m = {}
            for ch in range(NCH + 1):
                prev = cur
                cur = emit_dma(ch) if ch < NCH else None
                ic = ch - 1
                for sl in range(A + 2):
                    if cur is not None and sl < A:
                        stage_A(cur, sl, warm=(ch == 0))
                    if prev is None:
                        continue
                    if sl == 0:
                        prow = psR.tile([A, CH], F32)
                    if sl < A:
                        pend_P[sl] = stage_P(prev, sl)
                    if 1 <= sl < A + 1:
                        pend_m[sl - 1] = stage_m(prev, sl - 1,
                                                 pend_P.pop(sl - 1),
                                                 drain=(ic == NCH - 1))
                    if sl >= 2:
                        stage_f(prev, sl - 2, pend_m.pop(sl - 2), prow)
                if prev is not None:
                    orow = orow_p.tile([A, CH], F32)
                    nc.scalar.activation(orow[:], prow[:], AF.Identity)
                    nc.sync.dma_start(out_e[:, ic * CH:(ic + 1) * CH], orow[:])

    nc.compile()
    return nc


def _onescol():
    rs = np.zeros((E, A * A), np.float32)
    for i in range(A):
        rs[:, i * A + i] = 1.0
    return rs


def _pack_wga(w):  # [A, ACT, E] -> [2*ACT, A*E], agent a at rows (a%2)*ACT
    out = np.zeros((2 * ACT, A * E), w.dtype)
    for a in range(A):
        out[(a % 2) * ACT:(a % 2 + 1) * ACT, a * E:(a + 1) * E] = w[a]
    return out


def _fold_weights(inputs):
    """Fold training-mode BN into first-layer weights; pack for device."""
    f32 = np.float32
    obs = np.asarray(inputs["observation_vector"], f32)
    act = np.asarray(inputs["action_vector"], f32)
    g_gamma = np.asarray(inputs["g_gamma"], np.float64)
    g_beta = np.asarray(inputs["g_beta"], np.float64)
    Wg = np.asarray(inputs["Wg"], np.float64)
    bg = np.asarray(inputs["bg"], np.float64)
    s_gamma = np.asarray(inputs["s_gamma"], np.float64)
    s_beta = np.asarray(inputs["s_beta"], np.float64)
    Ws = np.asarray(inputs["Ws"], np.float64)
    bs = np.asarray(inputs["bs"], np.float64)

    mean_o = obs.mean(axis=1, dtype=np.float64)
    var_o = obs.var(axis=1, dtype=np.float64)
    mean_a = act.mean(axis=1, dtype=np.float64)
    var_a = act.var(axis=1, dtype=np.float64)

    sc_go = g_gamma[:, :OBS] / np.sqrt(var_o + EPS)
    sc_ga = g_gamma[:, OBS:] / np.sqrt(var_a + EPS)
    off_g = np.concatenate([g_beta[:, :OBS] - mean_o * sc_go,
                            g_beta[:, OBS:] - mean_a * sc_ga], axis=1)
    Wg_f = Wg * np.concatenate([sc_go, sc_ga], axis=1)[:, :, None]
    bg_f = bg + np.einsum("af,afe->ae", off_g, Wg)

    sc_s = s_gamma / np.sqrt(var_o + EPS)
    off_s = s_beta - mean_o * sc_s
    Ws_f = Ws * sc_s[:, :, None]
    bs_f = bs + np.einsum("af,afe->ae", off_s, Ws)

    bf16 = ml_dtypes.bfloat16
    Wq = np.asarray(inputs["Wq"], f32)
    Wk = np.asarray(inputs["Wk"], f32)
    Wv = np.asarray(inputs["Wv"], f32)
    Wf1 = np.asarray(inputs["Wf1"], np.float64)

    def packA(w):  # [A, R, E] -> [R, A*E]
        return np.ascontiguousarray(
            w.transpose(1, 0, 2).reshape(w.shape[1], -1))

    wf1x_np = packA((Wf1[:, :E, :] / (np.sqrt(D) * FP8_SCALE)).astype(f32))
    wf1 = np.concatenate([wf1x_np, packA(Wf1[:, E:, :].astype(f32))], axis=1)
    # DR layout: agent i block = [wf1x_i, wf1x_i] (each [E, E]), fp8
    wf1dr = np.empty((E, 2 * A * E), f32)
    for a in range(A):
        blk = wf1x_np[:, a * E:(a + 1) * E]
        wf1dr[:, 2 * a * E:(2 * a + 1) * E] = blk
        wf1dr[:, (2 * a + 1) * E:(2 * a + 2) * E] = blk

    bias_all = np.concatenate([
        bg_f.T, bs_f.T, np.asarray(inputs["bf1"], f32).T], axis=1)

    w = {
        "wgo": packA(Wg_f[:, :OBS, :].astype(f32)).astype(bf16),
        "wga": _pack_wga(Wg_f[:, OBS:, :].astype(f32)).astype(bf16),
        "ws": packA(Ws_f.astype(f32)).astype(bf16),
        "wqkv": np.concatenate([
            np.ascontiguousarray(Wq.transpose(1, 0, 2).reshape(E, E)),
            np.ascontiguousarray(Wk.transpose(1, 0, 2).reshape(E, E)),
            np.ascontiguousarray(Wv.transpose(1, 0, 2).reshape(E, E))],
            axis=1).astype(bf16),
        "wf1": wf1.astype(bf16),
        "wf1dr": wf1dr.astype(ml_dtypes.float8_e4m3),
        "delta": (FP8_SCALE * np.kron(np.eye(H, dtype=f32),
                                      np.ones((D, D), f32))).astype(bf16),
        "onescol": _onescol().astype(bf16),
        "bias_all": np.ascontiguousarray(bias_all.astype(f32)),
    }
    return w, obs, act


def make_in_maps(inputs, Bs):
    w, obs, act = _fold_weights(inputs)
    bf16 = ml_dtypes.bfloat16
    ids = np.argmax(np.asarray(inputs["action_vector"], np.float32), axis=2)
    Wf2 = np.asarray(inputs["Wf2"], np.float32)  # [A, E, ACT]
    # w2oh[a, e, b] = Wf2[a, e, ids[a, b]]
    w2oh = np.take_along_axis(
        Wf2, ids[:, None, :], axis=2)  # [A, E, B]

    in_maps = []
    for c in range(NCORES):
        sl = slice(c * Bs, (c + 1) * Bs)
        m = dict(w)
        m["obs_T"] = np.ascontiguousarray(
            obs[:, sl, :].transpose(2, 0, 1)).astype(bf16)
        def pack4(x):  # [A, Bs, ACT] -> [2*ACT, 4, Bs]
            t = x.transpose(2, 0, 1)  # [ACT, A, Bs]
            t2 = np.empty((2 * ACT, 4, t.shape[2]), t.dtype)
            for a in range(A):
                t2[(a % 2) * ACT:(a % 2 + 1) * ACT, a // 2] = t[:, a]
            return t2
        m["act_T"] = pack4(act[:, sl, :]).astype(bf16)
        m["w2oh_T"] = np.ascontiguousarray(
            w2oh[:, :, sl].transpose(1, 0, 2)).astype(bf16)
        in_maps.append(m)
    bf2 = np.asarray(inputs["bf2"], np.float32)
    host_bias = np.take_along_axis(
        np.broadcast_to(bf2[:, None, :], (A, B, ACT)), ids[:, :, None],
        axis=2)[:, :, 0]  # [A, B]
    return in_maps, host_bias


_NC_CACHE = {}


def run(inputs, trace=False, cfg=None, **kw):
    Bs = B // NCORES
    in_maps, host_bias = make_in_maps(inputs, Bs)
    key = (Bs, 512)
    if key not in _NC_CACHE:
        _NC_CACHE[key] = build_nc(Bs, 512, cfg)
    nc = _NC_CACHE[key]
    res = run_bass_kernel_spmd(nc, in_maps, core_ids=list(range(NCORES)),
                               trace=trace, **kw)
    outs = [r["out"] for r in res.results]
    full = np.concatenate(outs, axis=1) + host_bias
    return full.reshape(A, B, 1).astype(np.float32), res


def kernel(**inputs):
    out, _ = run(inputs, trace=False)
    return out


if __name__ == "__main__":
    print("kernel2 loaded")
